# revision 1
# baseline (speedup 1.0000x reference)
"""ChatGLM3 attention block on 8 Trainium2 NeuronCores.

Strategy: tensor-parallel over heads (TP-8). Core c computes q heads
4c..4c+3 plus the kv head c//4 (GQA: 2 kv heads, replicated 4x), runs the
full attention + its slice of the dense projection for both batches, and
returns a bf16 partial of the output. The host sums the 8 partials
(the "all-reduce after dense" step of the TP pattern, done at gather).

Device dataflow (per core, all matmul inputs bf16, fp32 PSUM accum):
  1. qkv projection: mixed^ = hidden @ w_qkv_slice.T + b (w pre-transposed
     and head-dim pair-deinterleaved on host; q rows pre-scaled by
     1/sqrt(hd)), in [token, channel] layout, 128-token tiles.
  2. GLM rope applied in natural layout (contiguous x1|x2 blocks thanks to
     the host-side pair de-interleave), then PE transposes to get
     qT/kT [head_dim, token]; v stays natural [token, head_dim].
  3. Attention per (batch, head): scoresT[j,i] = kT_j . qT_i via PE
     (contraction over head_dim on partitions), causal handled by skipping
     fully-masked j-tiles and adding -1e4 masks on diagonal tiles. exp on
     ScalarE (no max subtraction needed: |logits| <~ 10), softmax sums via
     all-ones matmul (broadcasts the j-sum to all 128 partitions),
     ctxT[d,i] accumulated with v as the stationary operand. Normalization
     folded into the PSUM->SBUF copy (reciprocal * ctx).
  4. Dense: out[t,o] partial = ctxT.T @ wd_slice^T with ctxT stationary,
     PSUM drained via ScalarE to bf16 staging, DMA'd out.
"""
import sys
import numpy as np

sys.path.insert(0, '/opt/trn_rl_repo')

import ml_dtypes

B, S, HIDDEN = 2, 2048, 4096
NH, NKV, HD = 32, 2, 128
ROT = HD // 2
THETA = 10000.0
SCALE = HD ** -0.5
N_CORES = 8
P = 128
NT = 32            # token tiles total (2 batches x 16)
NTB = 16           # token tiles per batch
CQ = 512           # q channels per core (4 heads)
CQK = 768          # q + k + v channels per core
NEG = -10000.0

bf16 = ml_dtypes.bfloat16

_cache = {}


def _build_program():
    from contextlib import ExitStack
    import concourse.bass as bass
    import concourse.mybir as mybir
    import concourse.tile as tile
    from concourse import bacc
    from concourse.bass import ts, ds
    from concourse.masks import make_identity

    dt = mybir.dt
    nc = bacc.Bacc("TRN2", debug=False, num_devices=N_CORES)

    hT_d = nc.dram_tensor("hT", [NT, P, 32, P], dt.bfloat16, kind="ExternalInput").ap()
    wq_d = nc.dram_tensor("wq", [P, 32, CQK], dt.bfloat16, kind="ExternalInput").ap()
    wd_d = nc.dram_tensor("wd", [P, 4, HIDDEN], dt.bfloat16, kind="ExternalInput").ap()
    tab_d = nc.dram_tensor("tab", [NT, P, 10, 32], dt.bfloat16, kind="ExternalInput").ap()
    mask_d = nc.dram_tensor("mask", [P, 4, 512], dt.bfloat16, kind="ExternalInput").ap()
    bias_d = nc.dram_tensor("bias", [1, CQK], dt.bfloat16, kind="ExternalInput").ap()
    out_d = nc.dram_tensor("out", [NT * P, HIDDEN], dt.bfloat16, kind="ExternalOutput").ap()

    EXP = mybir.ActivationFunctionType.Exp

    with tile.TileContext(nc) as tc:
        with ExitStack() as ctx:
            const = ctx.enter_context(tc.tile_pool(name="const", bufs=1))
            hT_pool = ctx.enter_context(tc.tile_pool(name="hT_pool", bufs=2))
            tab_pool = ctx.enter_context(tc.tile_pool(name="tab_pool", bufs=2))
            qk_pool = ctx.enter_context(tc.tile_pool(name="qk_pool", bufs=2))
            rt_pool = ctx.enter_context(tc.tile_pool(name="rt_pool", bufs=4))
            ex_pool = ctx.enter_context(tc.tile_pool(name="ex_pool", bufs=3))
            rc_pool = ctx.enter_context(tc.tile_pool(name="rc_pool", bufs=2))
            st_pool = ctx.enter_context(tc.tile_pool(name="st_pool", bufs=2))
            ps = ctx.enter_context(tc.tile_pool(name="ps", bufs=1, space="PSUM"))

            # --- resident tensors ---
            wq_sb = const.tile([P, 32, CQK], dt.bfloat16)
            nc.sync.dma_start(wq_sb[:], wq_d[:])
            wd_sb = const.tile([P, 4, HIDDEN], dt.bfloat16)
            nc.sync.dma_start(wd_sb[:], wd_d[:])
            mask_sb = const.tile([P, 4, 512], dt.bfloat16)
            nc.sync.dma_start(mask_sb[:], mask_d[:])
            bias_sb = const.tile([1, CQK], dt.bfloat16)
            nc.sync.dma_start(bias_sb[:], bias_d[:])
            ones_sb = const.tile([P, P], dt.bfloat16)
            nc.vector.memset(ones_sb[:], 1.0)
            ident_sb = const.tile([P, P], dt.bfloat16)
            make_identity(nc, ident_sb[:])

            qT_all = const.tile([P, 4, NT * P], dt.bfloat16)
            kT_all = const.tile([P, NT * P], dt.bfloat16)
            v_all = const.tile([P, NT, P], dt.bfloat16)
            ctxT_all = const.tile([P, 4, NT * P], dt.bfloat16)

            # --- phase 1: qkv projection + rope + transposes ---
            for t in range(NT):
                hT_t = hT_pool.tile([P, 32, P], dt.bfloat16)
                nc.sync.dma_start(hT_t[:], hT_d[t])
                tab_t = tab_pool.tile([P, 10, 32], dt.bfloat16)
                nc.sync.dma_start(tab_t[:], tab_d[t])

                m5 = ps.tile([P, 4, P], dt.float32, tag="m5", bufs=1)
                m2 = ps.tile([P, 256], dt.float32, tag="m2", bufs=1)
                nc.tensor.matmul(m5[:], ones_sb[0:1, :], bias_d_slice0 := bias_sb[:, 0:512],
                                 start=True, stop=False)
                nc.tensor.matmul(m2[:], ones_sb[0:1, :], bias_sb[:, 512:768],
                                 start=True, stop=False)
                for n in range(32):
                    lhsT = hT_t[:, n, :]
                    nc.tensor.matmul(m5[:], lhsT, wq_sb[:, n, 0:512],
                                     start=False, stop=(n == 31))
                    nc.tensor.matmul(m2[:], lhsT, wq_sb[:, n, 512:768],
                                     start=False, stop=(n == 31))

                qk_t = qk_pool.tile([P, 5, P], dt.bfloat16)
                # rope q (4 heads): x1' = x1*cos - x2*sin ; x2' = x1*sin + x2*cos
                x1, x2 = m5[:, :, 0:32], m5[:, :, 32:64]
                cq, sq = tab_t[:, 0:4, :], tab_t[:, 5:9, :]
                t1 = rt_pool.tile([P, 4, 32], dt.float32, tag="rt")
                nc.vector.tensor_mul(t1[:], x1, cq)
                t2 = rt_pool.tile([P, 4, 32], dt.float32, tag="rt")
                nc.vector.tensor_mul(t2[:], x2, sq)
                nc.vector.tensor_sub(qk_t[:, 0:4, 0:32], t1[:], t2[:])
                t3 = rt_pool.tile([P, 4, 32], dt.float32, tag="rt")
                nc.vector.tensor_mul(t3[:], x1, sq)
                t4 = rt_pool.tile([P, 4, 32], dt.float32, tag="rt")
                nc.vector.tensor_mul(t4[:], x2, cq)
                nc.vector.tensor_add(qk_t[:, 0:4, 32:64], t3[:], t4[:])
                nc.scalar.copy(qk_t[:, 0:4, 64:128], m5[:, :, 64:128])
                # rope k
                x1k, x2k = m2[:, 0:32], m2[:, 32:64]
                ck, sk = tab_t[:, 4, :], tab_t[:, 9, :]
                t5 = rt_pool.tile([P, 32], dt.float32, tag="rtk")
                nc.vector.tensor_mul(t5[:], x1k, ck)
                t6 = rt_pool.tile([P, 32], dt.float32, tag="rtk")
                nc.vector.tensor_mul(t6[:], x2k, sk)
                nc.vector.tensor_sub(qk_t[:, 4, 0:32], t5[:], t6[:])
                t7 = rt_pool.tile([P, 32], dt.float32, tag="rtk")
                nc.vector.tensor_mul(t7[:], x1k, sk)
                t8 = rt_pool.tile([P, 32], dt.float32, tag="rtk")
                nc.vector.tensor_mul(t8[:], x2k, ck)
                nc.vector.tensor_add(qk_t[:, 4, 32:64], t7[:], t8[:])
                nc.scalar.copy(qk_t[:, 4, 64:128], m2[:, 64:128])
                # v (no rope)
                nc.scalar.copy(v_all[:, t, :], m2[:, 128:256])

                for hb in range(5):
                    trp = ps.tile([P, P], dt.bfloat16, tag="tr", bufs=2)
                    nc.tensor.transpose(trp[:], qk_t[:, hb, :], ident_sb[:])
                    if hb < 4:
                        nc.scalar.copy(qT_all[:, hb, ts(t, P)], trp[:])
                    else:
                        nc.scalar.copy(kT_all[:, ts(t, P)], trp[:])

            # --- phase 2: attention ---
            for b in range(B):
                for h in range(4):
                    for ic in range(4):
                        icol = b * S + ic * 512
                        last = 4 * ic + 3
                        su = ps.tile([P, 512], dt.float32, tag="su", bufs=1)
                        cx = ps.tile([P, 512], dt.float32, tag="cx", bufs=1)
                        for jt in range(last + 1):
                            sc = ps.tile([P, 512], dt.float32, tag="sc", bufs=2)
                            nc.tensor.matmul(sc[:], kT_all[:, ts(b * NTB + jt, P)],
                                             qT_all[:, h, ds(icol, 512)],
                                             start=True, stop=True)
                            if jt >= 4 * ic:
                                nc.vector.tensor_add(sc[:], sc[:],
                                                     mask_sb[:, jt - 4 * ic, :])
                            ex = ex_pool.tile([P, 512], dt.bfloat16, tag="ex")
                            nc.scalar.activation(ex[:], sc[:], EXP)
                            nc.tensor.matmul(su[:], ones_sb[:], ex[:],
                                             start=(jt == 0), stop=(jt == last))
                            nc.tensor.matmul(cx[:], v_all[:, b * NTB + jt, :], ex[:],
                                             start=(jt == 0), stop=(jt == last))
                        rc = rc_pool.tile([P, 512], dt.float32, tag="rc")
                        nc.vector.reciprocal(rc[:], su[:])
                        nc.vector.tensor_mul(ctxT_all[:, h, ds(icol, 512)],
                                             cx[:], rc[:])

            # --- phase 3: dense partial ---
            DENSE_TAGS = [("m5", 1), ("m2", 1), ("sc", 2), ("sc", 2),
                          ("su", 1), ("cx", 1), ("tr", 2), ("tr", 2)]
            for t in range(NT):
                for half in range(2):
                    pos = []
                    for i in range(4):
                        tag, nb = DENSE_TAGS[half * 4 + i]
                        po = ps.tile([P, 512], dt.float32, tag=tag, bufs=nb)
                        pos.append(po)
                    for f in range(4):
                        lhsT = ctxT_all[:, f, ts(t, P)]
                        for i in range(4):
                            oc = half * 4 + i
                            nc.tensor.matmul(pos[i][:], lhsT,
                                             wd_sb[:, f, ds(oc * 512, 512)],
                                             start=(f == 0), stop=(f == 3))
                    stg = st_pool.tile([P, 2048], dt.bfloat16, tag="st")
                    for i in range(4):
                        nc.scalar.copy(stg[:, ts(i, 512)], pos[i][:])
                    nc.sync.dma_start(out_d[ts(t, P), ds(half * 2048, 2048)], stg[:])

    nc.compile()
    return nc


def _prep_core_inputs(c, hidden_bf, w_qkv, b_qkv, w_dense, tabs, masks, hT):
    perm = np.concatenate([np.arange(0, ROT, 2), np.arange(1, ROT, 2),
                           np.arange(ROT, HD)])
    kvh = c // 4
    q_rows = np.concatenate([h * HD + perm for h in range(4 * c, 4 * c + 4)])
    k_rows = NH * HD + kvh * HD + perm
    v_rows = NH * HD + NKV * HD + kvh * HD + np.arange(HD)
    rows = np.concatenate([q_rows, k_rows, v_rows])
    W = w_qkv[rows].astype(np.float32).copy()
    bias = b_qkv[rows].astype(np.float32).copy()
    W[:CQ] *= SCALE
    bias[:CQ] *= SCALE
    wq = np.ascontiguousarray(
        W.T.reshape(32, P, CQK).transpose(1, 0, 2)).astype(bf16)
    wd = np.ascontiguousarray(
        w_dense[:, c * CQ:(c + 1) * CQ].T.reshape(4, P, HIDDEN)
        .transpose(1, 0, 2)).astype(bf16)
    return {
        "hT": hT,
        "wq": wq,
        "wd": wd,
        "tab": tabs,
        "mask": masks,
        "bias": bias.reshape(1, CQK).astype(bf16),
    }


def _prep_shared(positions, hidden_states):
    hidden_bf = hidden_states.astype(bf16)
    # hT tiles [t, p, n, tok]
    hT = np.empty((NT, P, 32, P), dtype=bf16)
    for b in range(B):
        blk = hidden_bf[b].reshape(NTB, P, 32, P)       # [tb, tok, n, p]
        hT[b * NTB:(b + 1) * NTB] = blk.transpose(0, 3, 2, 1)
    # rope tables [t, p, 10, 32]: cosq x4 | cosk | sinq x4 | sink
    inv_freq = 1.0 / (THETA ** (np.arange(0, ROT, 2, dtype=np.float32) / ROT))
    tabs = np.empty((NT, P, 10, 32), dtype=bf16)
    for b in range(B):
        ang = positions[b].astype(np.float32)[:, None] * inv_freq   # [S, 32]
        cos, sin = np.cos(ang), np.sin(ang)
        for tb in range(NTB):
            sl = slice(tb * P, (tb + 1) * P)
            t = b * NTB + tb
            tabs[t, :, 0:4, :] = cos[sl][:, None, :]
            tabs[t, :, 4, :] = cos[sl]
            tabs[t, :, 5:9, :] = sin[sl][:, None, :]
            tabs[t, :, 9, :] = sin[sl]
    # causal masks for diagonal j-tiles [j, delta, i]
    j = np.arange(P)[:, None]
    i = np.arange(512)[None, :]
    masks = np.empty((P, 4, 512), dtype=bf16)
    for d in range(4):
        masks[:, d, :] = np.where(i >= d * P + j, 0.0, NEG).astype(bf16)
    return hidden_bf, hT, tabs, masks


def kernel(positions, hidden_states, w_qkv, b_qkv, w_dense):
    from concourse.bass_utils import run_bass_kernel_spmd

    positions = np.asarray(positions)
    hidden_states = np.asarray(hidden_states, dtype=np.float32)
    w_qkv = np.asarray(w_qkv, dtype=np.float32)
    b_qkv = np.asarray(b_qkv, dtype=np.float32)
    w_dense = np.asarray(w_dense, dtype=np.float32)

    if "nc" not in _cache:
        _cache["nc"] = _build_program()
    nc = _cache["nc"]

    hidden_bf, hT, tabs, masks = _prep_shared(positions, hidden_states)
    in_maps = [_prep_core_inputs(c, hidden_bf, w_qkv, b_qkv, w_dense,
                                 tabs, masks, hT) for c in range(N_CORES)]
    res = run_bass_kernel_spmd(nc, in_maps, list(range(N_CORES)))
    out = np.zeros((NT * P, HIDDEN), dtype=np.float32)
    for c in range(N_CORES):
        out += res.results[c]["out"].astype(np.float32)
    return out.reshape(B, S, HIDDEN)


# revision 4
# speedup vs baseline: 28.0437x; 28.0437x over previous
"""ChatGLM3 attention block on 8 Trainium2 NeuronCores.

Strategy: tensor-parallel over heads (TP-8). Core c computes q heads
4c..4c+3 plus the kv head c//4 (GQA: 2 kv heads, replicated 4x), runs the
full attention + its slice of the dense projection for both batches, and
returns a bf16 partial of the output. The host sums the 8 partials
(the "all-reduce after dense" step of the TP pattern, done at gather).

Device dataflow (per core, all matmul inputs bf16, fp32 PSUM accum):
  1. qkv projection: mixed^ = hidden @ w_qkv_slice.T + b (w pre-transposed
     and head-dim pair-deinterleaved on host; q rows pre-scaled by
     1/sqrt(hd)), in [token, channel] layout, 128-token tiles.
  2. GLM rope applied in natural layout (contiguous x1|x2 blocks thanks to
     the host-side pair de-interleave), then PE transposes to get
     qT/kT [head_dim, token]; v stays natural [token, head_dim].
  3. Attention per (batch, head): scoresT[j,i] = kT_j . qT_i via PE
     (contraction over head_dim on partitions), causal handled by skipping
     fully-masked j-tiles and adding -1e4 masks on diagonal tiles. exp on
     ScalarE (no max subtraction needed: |logits| <~ 10), softmax sums via
     all-ones matmul (broadcasts the j-sum to all 128 partitions),
     ctxT[d,i] accumulated with v as the stationary operand. Normalization
     folded into the PSUM->SBUF copy (reciprocal * ctx).
  4. Dense: out[t,o] partial = ctxT.T @ wd_slice^T with ctxT stationary,
     PSUM drained via ScalarE to bf16 staging, DMA'd out.
"""
import sys
import numpy as np

sys.path.insert(0, '/opt/trn_rl_repo')

import ml_dtypes

B, S, HIDDEN = 2, 2048, 4096
NH, NKV, HD = 32, 2, 128
ROT = HD // 2
THETA = 10000.0
SCALE = HD ** -0.5
N_CORES = 8
P = 128
NT = 32            # token tiles total (2 batches x 16)
NTB = 16           # token tiles per batch
CQ = 512           # q channels per core (4 heads)
CQK = 768          # q + k + v channels per core
NEG = -10000.0

bf16 = ml_dtypes.bfloat16

_cache = {}


def _build_program(reps=1):
    from contextlib import ExitStack
    import concourse.bass as bass
    import concourse.mybir as mybir
    import concourse.tile as tile
    from concourse import bacc
    from concourse.bass import ts, ds
    from concourse.masks import make_identity

    dt = mybir.dt
    nc = bacc.Bacc("TRN2", debug=False, num_devices=N_CORES)

    hT_d = nc.dram_tensor("hT", [NT, P, 32, P], dt.bfloat16, kind="ExternalInput").ap()
    wq_d = nc.dram_tensor("wq", [P, 32, CQK], dt.bfloat16, kind="ExternalInput").ap()
    wd_d = nc.dram_tensor("wd", [P, 4, HIDDEN], dt.bfloat16, kind="ExternalInput").ap()
    tab_d = nc.dram_tensor("tab", [NT, P, 10, 32], dt.bfloat16, kind="ExternalInput").ap()
    mask_d = nc.dram_tensor("mask", [P, 4, 512], dt.bfloat16, kind="ExternalInput").ap()
    bias_d = nc.dram_tensor("bias", [1, CQK], dt.bfloat16, kind="ExternalInput").ap()
    out_d = nc.dram_tensor("out", [NT * P, HIDDEN], dt.bfloat16, kind="ExternalOutput").ap()

    EXP = mybir.ActivationFunctionType.Exp

    with tile.TileContext(nc) as tc:
        with ExitStack() as ctx:
            const = ctx.enter_context(tc.tile_pool(name="const", bufs=1))
            hT_pool = ctx.enter_context(tc.tile_pool(name="hT_pool", bufs=2))
            tab_pool = ctx.enter_context(tc.tile_pool(name="tab_pool", bufs=2))
            qk_pool = ctx.enter_context(tc.tile_pool(name="qk_pool", bufs=2))
            rt_pool = ctx.enter_context(tc.tile_pool(name="rt_pool", bufs=4))
            ex_pool = ctx.enter_context(tc.tile_pool(name="ex_pool", bufs=3))
            rc_pool = ctx.enter_context(tc.tile_pool(name="rc_pool", bufs=2))
            st_pool = ctx.enter_context(tc.tile_pool(name="st_pool", bufs=2))
            ps = ctx.enter_context(tc.tile_pool(name="ps", bufs=1, space="PSUM"))

            # --- resident tensors ---
            wq_sb = const.tile([P, 32, CQK], dt.bfloat16)
            nc.sync.dma_start(wq_sb[:], wq_d[:])
            wd_sb = const.tile([P, 4, HIDDEN], dt.bfloat16)
            nc.sync.dma_start(wd_sb[:], wd_d[:])
            mask_sb = const.tile([P, 4, 512], dt.bfloat16)
            nc.sync.dma_start(mask_sb[:], mask_d[:])
            bias_sb = const.tile([1, CQK], dt.bfloat16)
            nc.sync.dma_start(bias_sb[:], bias_d[:])
            ones_sb = const.tile([P, P], dt.bfloat16)
            nc.vector.memset(ones_sb[:], 1.0)
            ident_sb = const.tile([P, P], dt.bfloat16)
            make_identity(nc, ident_sb[:])

            qT_all = const.tile([P, 4, NT * P], dt.bfloat16)
            kT_all = const.tile([P, NT * P], dt.bfloat16)
            v_all = const.tile([P, NT, P], dt.bfloat16)
            ctxT_all = const.tile([P, 4, NT * P], dt.bfloat16)

            def _emit_body():
                # --- phase 1: qkv projection + rope + transposes ---
                for t in range(NT):
                    hT_t = hT_pool.tile([P, 32, P], dt.bfloat16)
                    nc.sync.dma_start(hT_t[:], hT_d[t])
                    tab_t = tab_pool.tile([P, 10, 32], dt.bfloat16)
                    nc.sync.dma_start(tab_t[:], tab_d[t])

                    m5 = ps.tile([P, 4, P], dt.float32, tag="m5", bufs=1)
                    m2 = ps.tile([P, 256], dt.float32, tag="m2", bufs=1)
                    nc.tensor.matmul(m5[:], ones_sb[0:1, :], bias_d_slice0 := bias_sb[:, 0:512],
                                     start=True, stop=False)
                    nc.tensor.matmul(m2[:], ones_sb[0:1, :], bias_sb[:, 512:768],
                                     start=True, stop=False)
                    for n in range(32):
                        lhsT = hT_t[:, n, :]
                        nc.tensor.matmul(m5[:], lhsT, wq_sb[:, n, 0:512],
                                         start=False, stop=(n == 31))
                        nc.tensor.matmul(m2[:], lhsT, wq_sb[:, n, 512:768],
                                         start=False, stop=(n == 31))

                    qk_t = qk_pool.tile([P, 5, P], dt.bfloat16)
                    # rope q (4 heads): x1' = x1*cos - x2*sin ; x2' = x1*sin + x2*cos
                    x1, x2 = m5[:, :, 0:32], m5[:, :, 32:64]
                    cq, sq = tab_t[:, 0:4, :], tab_t[:, 5:9, :]
                    t1 = rt_pool.tile([P, 4, 32], dt.float32, tag="rt")
                    nc.vector.tensor_mul(t1[:], x1, cq)
                    t2 = rt_pool.tile([P, 4, 32], dt.float32, tag="rt")
                    nc.vector.tensor_mul(t2[:], x2, sq)
                    nc.vector.tensor_sub(qk_t[:, 0:4, 0:32], t1[:], t2[:])
                    t3 = rt_pool.tile([P, 4, 32], dt.float32, tag="rt")
                    nc.vector.tensor_mul(t3[:], x1, sq)
                    t4 = rt_pool.tile([P, 4, 32], dt.float32, tag="rt")
                    nc.vector.tensor_mul(t4[:], x2, cq)
                    nc.vector.tensor_add(qk_t[:, 0:4, 32:64], t3[:], t4[:])
                    nc.scalar.copy(qk_t[:, 0:4, 64:128], m5[:, :, 64:128])
                    # rope k
                    x1k, x2k = m2[:, 0:32], m2[:, 32:64]
                    ck, sk = tab_t[:, 4, :], tab_t[:, 9, :]
                    t5 = rt_pool.tile([P, 32], dt.float32, tag="rtk")
                    nc.vector.tensor_mul(t5[:], x1k, ck)
                    t6 = rt_pool.tile([P, 32], dt.float32, tag="rtk")
                    nc.vector.tensor_mul(t6[:], x2k, sk)
                    nc.vector.tensor_sub(qk_t[:, 4, 0:32], t5[:], t6[:])
                    t7 = rt_pool.tile([P, 32], dt.float32, tag="rtk")
                    nc.vector.tensor_mul(t7[:], x1k, sk)
                    t8 = rt_pool.tile([P, 32], dt.float32, tag="rtk")
                    nc.vector.tensor_mul(t8[:], x2k, ck)
                    nc.vector.tensor_add(qk_t[:, 4, 32:64], t7[:], t8[:])
                    nc.scalar.copy(qk_t[:, 4, 64:128], m2[:, 64:128])
                    # v (no rope)
                    nc.scalar.copy(v_all[:, t, :], m2[:, 128:256])

                    for hb in range(5):
                        trp = ps.tile([P, P], dt.bfloat16, tag="tr", bufs=2)
                        nc.tensor.transpose(trp[:], qk_t[:, hb, :], ident_sb[:])
                        if hb < 4:
                            nc.scalar.copy(qT_all[:, hb, ts(t, P)], trp[:])
                        else:
                            nc.scalar.copy(kT_all[:, ts(t, P)], trp[:])

                # --- phase 2: attention ---
                # Software-pipelined: the sc matmul for jt+1 is issued to the
                # PE before the exp-dependent su/cx of jt, so the PE streams
                # the next score tile while ScalarE runs exp on the current.
                for b in range(B):
                    for h in range(4):
                        for ic in range(4):
                            icol = b * S + ic * 512
                            last = 4 * ic + 3
                            su = ps.tile([P, 512], dt.float32, tag="su", bufs=1)
                            cx = ps.tile([P, 512], dt.float32, tag="cx", bufs=1)

                            def issue_sc(jt):
                                sc = ps.tile([P, 512], dt.float32, tag="sc",
                                             bufs=2)
                                nc.tensor.matmul(sc[:],
                                                 kT_all[:, ts(b * NTB + jt, P)],
                                                 qT_all[:, h, ds(icol, 512)],
                                                 start=True, stop=True)
                                if jt >= 4 * ic:
                                    nc.vector.tensor_add(sc[:], sc[:],
                                                         mask_sb[:, jt - 4 * ic, :])
                                return sc

                            def consume(jt, sc):
                                ex = ex_pool.tile([P, 512], dt.bfloat16, tag="ex")
                                nc.scalar.activation(ex[:], sc[:], EXP)
                                nc.tensor.matmul(su[:], ones_sb[:], ex[:],
                                                 start=(jt == 0), stop=(jt == last))
                                nc.tensor.matmul(cx[:], v_all[:, b * NTB + jt, :],
                                                 ex[:],
                                                 start=(jt == 0), stop=(jt == last))

                            sc_prev = issue_sc(0)
                            for jt in range(1, last + 1):
                                sc_cur = issue_sc(jt)
                                consume(jt - 1, sc_prev)
                                sc_prev = sc_cur
                            consume(last, sc_prev)
                            rc = rc_pool.tile([P, 512], dt.float32, tag="rc")
                            nc.vector.reciprocal(rc[:], su[:])
                            nc.vector.tensor_mul(ctxT_all[:, h, ds(icol, 512)],
                                                 cx[:], rc[:])

                # --- phase 3: dense partial ---
                DENSE_TAGS = [("m5", 1), ("m2", 1), ("sc", 2), ("sc", 2),
                              ("su", 1), ("cx", 1), ("tr", 2), ("tr", 2)]
                for t in range(NT):
                    for half in range(2):
                        pos = []
                        for i in range(4):
                            tag, nb = DENSE_TAGS[half * 4 + i]
                            po = ps.tile([P, 512], dt.float32, tag=tag, bufs=nb)
                            pos.append(po)
                        for f in range(4):
                            lhsT = ctxT_all[:, f, ts(t, P)]
                            for i in range(4):
                                oc = half * 4 + i
                                nc.tensor.matmul(pos[i][:], lhsT,
                                                 wd_sb[:, f, ds(oc * 512, 512)],
                                                 start=(f == 0), stop=(f == 3))
                        stg = st_pool.tile([P, 2048], dt.bfloat16, tag="st")
                        for i in range(4):
                            nc.scalar.copy(stg[:, ts(i, 512)], pos[i][:])
                        nc.sync.dma_start(out_d[ts(t, P), ds(half * 2048, 2048)], stg[:])

            if reps > 1:
                with tc.For_i(0, reps, 1):
                    _emit_body()
            else:
                _emit_body()

    nc.compile()
    return nc


def _prep_core_inputs(c, hidden_bf, w_qkv, b_qkv, w_dense, tabs, masks, hT):
    perm = np.concatenate([np.arange(0, ROT, 2), np.arange(1, ROT, 2),
                           np.arange(ROT, HD)])
    kvh = c // 4
    q_rows = np.concatenate([h * HD + perm for h in range(4 * c, 4 * c + 4)])
    k_rows = NH * HD + kvh * HD + perm
    v_rows = NH * HD + NKV * HD + kvh * HD + np.arange(HD)
    rows = np.concatenate([q_rows, k_rows, v_rows])
    W = w_qkv[rows].astype(np.float32).copy()
    bias = b_qkv[rows].astype(np.float32).copy()
    W[:CQ] *= SCALE
    bias[:CQ] *= SCALE
    wq = np.ascontiguousarray(
        W.T.reshape(32, P, CQK).transpose(1, 0, 2)).astype(bf16)
    wd = np.ascontiguousarray(
        w_dense[:, c * CQ:(c + 1) * CQ].T.reshape(4, P, HIDDEN)
        .transpose(1, 0, 2)).astype(bf16)
    return {
        "hT": hT,
        "wq": wq,
        "wd": wd,
        "tab": tabs,
        "mask": masks,
        "bias": bias.reshape(1, CQK).astype(bf16),
    }


def _prep_shared(positions, hidden_states):
    hidden_bf = hidden_states.astype(bf16)
    # hT tiles [t, p, n, tok]
    hT = np.empty((NT, P, 32, P), dtype=bf16)
    for b in range(B):
        blk = hidden_bf[b].reshape(NTB, P, 32, P)       # [tb, tok, n, p]
        hT[b * NTB:(b + 1) * NTB] = blk.transpose(0, 3, 2, 1)
    # rope tables [t, p, 10, 32]: cosq x4 | cosk | sinq x4 | sink
    inv_freq = 1.0 / (THETA ** (np.arange(0, ROT, 2, dtype=np.float32) / ROT))
    tabs = np.empty((NT, P, 10, 32), dtype=bf16)
    for b in range(B):
        ang = positions[b].astype(np.float32)[:, None] * inv_freq   # [S, 32]
        cos, sin = np.cos(ang), np.sin(ang)
        for tb in range(NTB):
            sl = slice(tb * P, (tb + 1) * P)
            t = b * NTB + tb
            tabs[t, :, 0:4, :] = cos[sl][:, None, :]
            tabs[t, :, 4, :] = cos[sl]
            tabs[t, :, 5:9, :] = sin[sl][:, None, :]
            tabs[t, :, 9, :] = sin[sl]
    # causal masks for diagonal j-tiles [j, delta, i]
    j = np.arange(P)[:, None]
    i = np.arange(512)[None, :]
    masks = np.empty((P, 4, 512), dtype=bf16)
    for d in range(4):
        masks[:, d, :] = np.where(i >= d * P + j, 0.0, NEG).astype(bf16)
    return hidden_bf, hT, tabs, masks


def kernel(positions, hidden_states, w_qkv, b_qkv, w_dense):
    from concourse.bass_utils import run_bass_kernel_spmd

    positions = np.asarray(positions)
    hidden_states = np.asarray(hidden_states, dtype=np.float32)
    w_qkv = np.asarray(w_qkv, dtype=np.float32)
    b_qkv = np.asarray(b_qkv, dtype=np.float32)
    w_dense = np.asarray(w_dense, dtype=np.float32)

    if "nc" not in _cache:
        _cache["nc"] = _build_program()
    nc = _cache["nc"]

    hidden_bf, hT, tabs, masks = _prep_shared(positions, hidden_states)
    in_maps = [_prep_core_inputs(c, hidden_bf, w_qkv, b_qkv, w_dense,
                                 tabs, masks, hT) for c in range(N_CORES)]
    res = run_bass_kernel_spmd(nc, in_maps, list(range(N_CORES)))
    out = np.zeros((NT * P, HIDDEN), dtype=np.float32)
    for c in range(N_CORES):
        out += res.results[c]["out"].astype(np.float32)
    return out.reshape(B, S, HIDDEN)

